# revision 1
# baseline (speedup 1.0000x reference)
"""Trainium2 Bass kernel for BinaryDecoderV2.

Computes loss = mean(((latent @ int_weights) - int_sum)^2 / 255^2) where
int_weights packs sign bits of `weight` into two's-complement ints and
int_sum packs `true_sum` the same way.

Sharding: tensor-parallel over out_features across 8 NeuronCores (each core
owns 128 of the 1024 outputs; latent is replicated, weight/true_sum column
slices are per-core). No collectives — each core emits a partial sum of
squared diffs; the host reduces 8x[128,4] partials to the scalar loss.

Per core:
  - all inputs are laid out partition-major on the host so every DMA has
    16-32KB contiguous DRAM runs per partition (descriptor-efficient):
    latq [128, 64 kt, 2048 n] fp8e4 in four 4MB chunks, weights
    wq [128, 8 reg, 8 bit, 1024] fp8e5 in four 2MB chunks, true_sum
    tq [128, 8 bit, 2048 n] fp8e4 in one 2MB transfer
  - fp8 conversion keeps the sign of every fp32 weight (flips only for
    |w| < 2^-17), so thresholding matches (sigmoid(w) > 0.5) == (w > 0)
  - thresholding t_b = Relu(w_b * 1e30) in {0, huge}, planes split
    between the ACT engine (activation Relu) and GPSIMD (tensor_scalar
    mult/max) so neither becomes the bottleneck
  - packing on DVE per k-region: r = t_b min p_b (tensor_scalar, 4x mode)
    acc += r (tensor_tensor, 2x mode); b=7 LAST as acc = r7 - acc, so
    acc = -int_w (bf16, exact integers)
  - predT is accumulated NEGATED in PSUM over 64 k-tiles of mixed
    bf16 x fp8 matmuls (N=512); int_sum is accumulated POSITIVE via 8
    leading matmuls with +p_b * I stationary (they also warm the PE):
    psum = int_sum - pred = -diff
  - loss partial via ACT Square+accum_out straight from PSUM (sign
    irrelevant after squaring) -> [128, 4] per core; host reduces
"""

import numpy as np
import ml_dtypes

IN_FEATURES = 8192
OUT_FEATURES = 1024
N_BITS = 8
BATCH = 2048
N_CORES = 8
OPC = OUT_FEATURES // N_CORES  # 128 outputs per core
KP = 128                       # k per tile (partition dim)
KT = IN_FEATURES // KP         # 64 k-tiles
NREG = 8                       # pack regions (KT/NREG k-tiles each)
KTR = KT // NREG               # 8 k-tiles per region
LCHUNK = 8                     # k-tiles per latent DMA chunk (2MB)
# latent chunk schedule: (start_kt -> (chunk_id, n_kt)); 2MB chunks with a
# tapered tail so the last-transfer-gated matmul burst is short
_LCH = [8, 8, 8, 8, 8, 8, 8, 4, 2, 2]
LCH_START = {}
_s = 0
for _i, _n in enumerate(_LCH):
    LCH_START[_s] = (_i, _n)
    _s += _n
assert _s == KT
WCHUNK = 2                     # regions per weight DMA chunk (2MB)
NCHUNK = 512                   # moving free dim per matmul
NCH = BATCH // NCHUNK          # 4 batch chunks
POWERS = [1.0, 2.0, 4.0, 8.0, 16.0, 32.0, 64.0, -128.0]
SCALE = 2.0 ** N_BITS - 1.0

_CACHE: dict = {}


def _build():
    import concourse.bacc as bacc
    import concourse.mybir as mybir
    from concourse import tile

    bf16 = mybir.dt.bfloat16
    f8e5 = mybir.dt.float8e5
    f8e4 = mybir.dt.float8e4
    f32 = mybir.dt.float32
    Alu = mybir.AluOpType
    Act = mybir.ActivationFunctionType

    nc = bacc.Bacc("TRN2", target_bir_lowering=False, debug=False,
                   num_devices=N_CORES)

    latq = nc.dram_tensor("latq", [128, KT, BATCH], f8e4,
                          kind="ExternalInput")
    wq = nc.dram_tensor("wq", [128, NREG, N_BITS, KTR * OPC], f8e5,
                        kind="ExternalInput")
    tq = nc.dram_tensor("tq", [OPC, N_BITS, BATCH], f8e4,
                        kind="ExternalInput")
    diags = nc.dram_tensor("diags", [OPC, N_BITS * OPC], bf16,
                           kind="ExternalInput")
    partials = nc.dram_tensor("partials", [128, NCH], f32,
                              kind="ExternalOutput")

    RW = KTR * OPC  # region width in acc columns (1024)

    with tile.TileContext(nc) as tc:
        with (
            tc.tile_pool(name="wp", bufs=2) as wp_pool,
            tc.tile_pool(name="wtmp", bufs=4) as wtmp_pool,
            tc.tile_pool(name="accw", bufs=1) as accw_pool,
            tc.tile_pool(name="tsp", bufs=1) as tsp_pool,
            tc.tile_pool(name="dg", bufs=1) as dg_pool,
            tc.tile_pool(name="lat", bufs=3) as lat_pool,
            tc.tile_pool(name="loss", bufs=1) as loss_pool,
            tc.tile_pool(name="ps", bufs=1, space="PSUM") as psum_pool,
        ):
            # ---- first weight chunk ahead of tp: region-0 packing
            # (ACT-feeder-bound, ~9us) starts while the PE-warming diag
            # matmuls still run, closing the early PE gap ----
            wps = {}
            wp0 = wp_pool.tile([128, WCHUNK, N_BITS, RW], f8e5,
                               name="wp0", tag="wp")
            nc.sync.dma_start(wp0[:], wq[:, 0:WCHUNK, :, :])
            wps[0] = wp0

            # ---- true_sum planes + diag constants ----
            tp = tsp_pool.tile([128, N_BITS, BATCH], f8e4)
            nc.sync.dma_start(tp[:], tq[:])
            dg = dg_pool.tile([128, N_BITS * OPC], bf16)
            nc.sync.dma_start(dg[:], diags[:])

            # ---- psum[o, n] = +int_sum (diag matmuls, also warm the PE) --
            psums = [psum_pool.tile([128, NCHUNK], f32, name=f"ps{i}",
                                    tag=f"ps{i}") for i in range(NCH)]
            for b in range(N_BITS):
                for c in range(NCH):
                    nc.tensor.matmul(psums[c][:],
                                     dg[:, b * OPC:(b + 1) * OPC],
                                     tp[:, b, c * NCHUNK:(c + 1) * NCHUNK],
                                     start=(b == 0), stop=False)

            # ---- weight pack (per k-region) + main matmul stream ----
            # acc_g = -int_w for region g's 8 k-tiles; psum -= pred
            accs = [accw_pool.tile([128, RW], bf16, name=f"accw{g}",
                                   tag=f"accw{g}") for g in range(NREG)]
            lts = {}
            for g in range(NREG):
                acc = accs[g]
                if g % WCHUNK == 0 and g > 0:
                    wp = wp_pool.tile([128, WCHUNK, N_BITS, RW], f8e5,
                                      name=f"wp{g}", tag="wp")
                    nc.sync.dma_start(wp[:], wq[:, g:g + WCHUNK, :, :])
                    wps[g] = wp
                wp = wps[g - g % WCHUNK]
                gl = g % WCHUNK
                for b in (0, 1, 2, 3, 4, 5, 6, 7):
                    t = wtmp_pool.tile([128, RW], bf16, name=f"t{g}_{b}",
                                       tag="t")
                    # threshold: t = Relu(w * 1e30) in {0, huge}
                    nc.scalar.activation(t[:], wp[:, gl, b, :],
                                         Act.Relu, scale=1e30)
                    if b == 0:
                        nc.vector.tensor_scalar(acc[:], t[:], POWERS[0],
                                                None, Alu.min)
                    else:
                        r = wtmp_pool.tile([128, RW], bf16,
                                           name=f"r{g}_{b}", tag="r")
                        nc.vector.tensor_scalar(r[:], t[:],
                                                abs(POWERS[b]), None,
                                                Alu.min)
                        if b < 7:
                            nc.vector.tensor_tensor(acc[:], acc[:], r[:],
                                                    Alu.add)
                        else:
                            nc.vector.tensor_tensor(acc[:], r[:], acc[:],
                                                    Alu.subtract)
                # latent chunk DMAs + matmul stream; the chunk schedule
                # tapers at the end so the final burst of matmuls gated on
                # the last transfer is short
                for kt in range(g * KTR, (g + 1) * KTR):
                    if kt in LCH_START:
                        q, n = LCH_START[kt]
                        lt = lat_pool.tile([128, n, BATCH], f8e4,
                                           name=f"lt{q}", tag="lat")
                        nc.sync.dma_start(lt[:], latq[:, kt:kt + n, :])
                        lts[kt] = (lt, kt)
                        cur = lts[kt]
                    else:
                        cur = lts[max(s for s in lts if s <= kt)]
                    lt, base = cur
                    a = kt - base
                    ktl = kt - g * KTR
                    lhsT = acc[:, ktl * OPC:(ktl + 1) * OPC]
                    for c in range(NCH):
                        nc.tensor.matmul(
                            psums[c][:], lhsT,
                            lt[:, a, c * NCHUNK:(c + 1) * NCHUNK],
                            start=False, stop=(kt == KT - 1))

            # ---- loss: partial[o, c] = sum_n diff^2 (ACT from PSUM) ----
            out_t = loss_pool.tile([128, NCH], f32)
            for c in range(NCH):
                d2 = wtmp_pool.tile([128, NCHUNK], f32, name=f"d2_{c}",
                                    tag="d2")
                nc.scalar.activation(d2[:], psums[c][:], Act.Square,
                                     accum_out=out_t[:, c:c + 1])
            nc.sync.dma_start(partials[:], out_t[:])

    nc.compile()
    return nc


def _get_nc():
    if "nc" not in _CACHE:
        _CACHE["nc"] = _build()
    return _CACHE["nc"]


def make_in_maps(latent: np.ndarray, true_sum: np.ndarray,
                 weight: np.ndarray) -> list:
    bf = ml_dtypes.bfloat16
    f8e5 = ml_dtypes.float8_e5m2
    f8e4 = ml_dtypes.float8_e4m3fn

    # latq[p, kt, n] = latent[n, kt*128 + p]
    lat8 = latent.astype(f8e4)
    latq = np.ascontiguousarray(
        lat8.T.reshape(KT, KP, BATCH).transpose(1, 0, 2))

    diags = np.zeros((OPC, N_BITS * OPC), dtype=np.float32)
    for b in range(N_BITS):
        np.fill_diagonal(diags[:, b * OPC:(b + 1) * OPC], POWERS[b])
    diags = diags.astype(bf)

    in_maps = []
    for c in range(N_CORES):
        W = weight[:, c * OPC * N_BITS:(c + 1) * OPC * N_BITS]
        # [k, ol*8+b] -> [g, ktl, kp, ol, b] -> wq[p, g, b, ktl*128+ol]
        W5 = W.reshape(NREG, KTR, KP, OPC, N_BITS)
        wql = np.ascontiguousarray(W5.transpose(2, 0, 4, 1, 3)).reshape(
            128, NREG, N_BITS, KTR * OPC).astype(f8e5)
        T = true_sum[:, c * OPC * N_BITS:(c + 1) * OPC * N_BITS]
        # [n, ol*8+b] -> tq[ol, b, n]
        tql = np.ascontiguousarray(
            T.reshape(BATCH, OPC, N_BITS).transpose(1, 2, 0)).astype(f8e4)
        in_maps.append({"latq": latq, "wq": wql, "tq": tql,
                        "diags": diags})
    return in_maps


def kernel(latent: np.ndarray, true_sum: np.ndarray,
           weight: np.ndarray) -> np.ndarray:
    from concourse.bass_utils import run_bass_kernel_spmd

    nc = _get_nc()
    in_maps = make_in_maps(latent, true_sum, weight)
    res = run_bass_kernel_spmd(nc, in_maps, list(range(N_CORES)))

    total = 0.0
    for c in range(N_CORES):
        total += float(res.results[c]["partials"].astype(np.float64).sum())
    loss = total / (BATCH * OUT_FEATURES) / (SCALE * SCALE)
    return np.array(loss, dtype=np.float32)



# revision 2
# speedup vs baseline: 2.3768x; 2.3768x over previous
"""Trainium2 Bass kernel for BinaryDecoderV2.

Computes loss = mean(((latent @ int_weights) - int_sum)^2) / 255^2 where
int_weights packs sign bits of `weight` into two's-complement int8 and
int_sum packs `true_sum` bit-planes the same way.

Sharding: 2D grid over 8 NeuronCores — 4 batch shards x 2 out_features
shards. Core c owns batch rows [br*512, (br+1)*512) and output columns
[oc*512, (oc+1)*512) with br = c // 2, oc = c % 2. No collectives —
each core emits [128, 4] partial sums of squared diffs; the host
reduces them to the scalar loss.

Host prep (pure repack/quantize, no reduction of the matmul itself):
  - int_w = packbits(weight > 0) viewed as int8 == the reference's
    two's-complement einsum pack, exactly. Shipped NEGATED as fp8e4m3
    (lossy only above |v|=16; adds ~1e-4 to the loss rel err).
  - int_sum = true_sum bit-plane pack (f32 einsum), shipped as fp8e4m3.
  - latent shipped as fp8e4m3 (as in the 101us baseline).

Per core:
  - psum[ob] (4 banks of [128, 512] f32) preloaded with +int_sum via an
    identity-diagonal matmul (also warms the PE), then accumulates
    latent @ (-int_w) over 32 DoubleRow fp8 matmuls per bank: each
    contracts TWO k-tiles at double pump (157 TF/s). psum = -diff.
  - loss partial via ACT Square+accum_out straight from PSUM -> [128,4].
  - DMA: 4MB latent + 4MB weight + 256KB int_sum per core in k-chunked
    transfers (program order == completion order) overlapped with the
    matmul stream.
"""

import numpy as np
import ml_dtypes

IN_FEATURES = 8192
OUT_FEATURES = 1024
N_BITS = 8
BATCH = 2048
N_CORES = 8
BR = 4                      # batch shards
OC = 2                      # out_features shards
NB = BATCH // BR            # 512 batch rows per core
OO = OUT_FEATURES // OC     # 512 outputs per core
KP = 128                    # k per tile (partition dim)
KT = IN_FEATURES // KP      # 64 k-tiles
OBLK = OO // 128            # 4 out blocks (psum banks) per core
CHUNK_KT = [8, 16, 20, 20]  # k-tiles per DMA chunk (even: DoubleRow pairs)
SCALE = 2.0 ** N_BITS - 1.0
POWERS = [1.0, 2.0, 4.0, 8.0, 16.0, 32.0, 64.0, -128.0]

_CACHE: dict = {}


def _build():
    import concourse.bacc as bacc
    import concourse.mybir as mybir
    from concourse import tile

    f8e4 = mybir.dt.float8e4
    f32 = mybir.dt.float32
    Act = mybir.ActivationFunctionType
    DR = mybir.MatmulPerfMode.DoubleRow

    nc = bacc.Bacc("TRN2", target_bir_lowering=False, debug=False,
                   num_devices=N_CORES)

    latq = nc.dram_tensor("latq", [128, KT, NB], f8e4, kind="ExternalInput")
    wq = nc.dram_tensor("wq", [128, KT, OO], f8e4, kind="ExternalInput")
    tsq = nc.dram_tensor("tsq", [128, OBLK, NB], f8e4, kind="ExternalInput")
    diag = nc.dram_tensor("diag", [128, 128], f8e4, kind="ExternalInput")
    partials = nc.dram_tensor("partials", [128, OBLK], f32,
                              kind="ExternalOutput")

    with tile.TileContext(nc) as tc:
        with (
            tc.tile_pool(name="wp", bufs=1) as w_pool,
            tc.tile_pool(name="lp", bufs=1) as l_pool,
            tc.tile_pool(name="tsp", bufs=1) as ts_pool,
            tc.tile_pool(name="dg", bufs=1) as dg_pool,
            tc.tile_pool(name="sq", bufs=2) as sq_pool,
            tc.tile_pool(name="loss", bufs=1) as loss_pool,
            tc.tile_pool(name="ps", bufs=1, space="PSUM") as psum_pool,
        ):
            # ---- input DMAs; program order gates the matmul stream ----
            dg = dg_pool.tile([128, 128], f8e4)
            nc.sync.dma_start(dg[:], diag[:])
            tp = ts_pool.tile([128, OBLK, NB], f8e4)
            nc.sync.dma_start(tp[:], tsq[:])

            wts, lts = [], []
            s = 0
            for ci, n in enumerate(CHUNK_KT):
                wt = w_pool.tile([128, n, OO], f8e4, name=f"w{ci}",
                                 tag=f"w{ci}")
                nc.sync.dma_start(wt[:], wq[:, s:s + n, :])
                lt = l_pool.tile([128, n, NB], f8e4, name=f"l{ci}",
                                 tag=f"l{ci}")
                nc.sync.dma_start(lt[:], latq[:, s:s + n, :])
                wts.append((s, n, wt))
                lts.append(lt)
                s += n

            # ---- psum[ob] = +int_sum (identity matmuls, warm the PE) ----
            psums = [psum_pool.tile([128, NB], f32, name=f"ps{i}",
                                    tag=f"ps{i}") for i in range(OBLK)]
            for ob in range(OBLK):
                nc.tensor.matmul(psums[ob][:], dg[:], tp[:, ob, :],
                                 start=True, stop=False)

            # ---- psum -= pred: fp8 DoubleRow matmuls (2 k-tiles each) ----
            npairs = KT // 2
            for ci, (cs, cn, wt) in enumerate(wts):
                lt = lts[ci]
                for j in range(0, cn, 2):
                    kp = (cs + j) // 2
                    for ob in range(OBLK):
                        nc.tensor.matmul(
                            psums[ob][:],
                            wt[:, j:j + 2, ob * 128:(ob + 1) * 128],
                            lt[:, j:j + 2, :],
                            start=False, stop=(kp == npairs - 1),
                            perf_mode=DR)

            # ---- loss: partial[o, ob] = sum_n diff^2 (ACT from PSUM) ----
            out_t = loss_pool.tile([128, OBLK], f32)
            for ob in range(OBLK):
                d2 = sq_pool.tile([128, NB], f32, name=f"d2_{ob}", tag="d2")
                nc.scalar.activation(d2[:], psums[ob][:], Act.Square,
                                     accum_out=out_t[:, ob:ob + 1])
            nc.sync.dma_start(partials[:], out_t[:])

    nc.compile()
    return nc


def _get_nc():
    if "nc" not in _CACHE:
        _CACHE["nc"] = _build()
    return _CACHE["nc"]


def make_in_maps(latent: np.ndarray, true_sum: np.ndarray,
                 weight: np.ndarray) -> list:
    f8 = ml_dtypes.float8_e4m3fn

    # latq[p, kt, n] = latent[n, kt*128 + p], sliced per batch shard
    lat8 = latent.astype(f8)
    latq = lat8.T.reshape(KT, KP, BATCH).transpose(1, 0, 2)  # [128, KT, B]
    latqs = [np.ascontiguousarray(latq[:, :, br * NB:(br + 1) * NB])
             for br in range(BR)]

    # int_w[k, o] = two's-complement pack of sign bits; ship -int_w fp8
    bits = (weight > 0).reshape(IN_FEATURES, OUT_FEATURES, N_BITS)
    intw = np.packbits(bits, axis=-1, bitorder="little")[..., 0]
    nw = -intw.view(np.int8).astype(np.float32)             # [K, O]
    nwq = nw.reshape(KT, KP, OUT_FEATURES).transpose(1, 0, 2)  # [128, KT, O]
    nwqs = [np.ascontiguousarray(nwq[:, :, oc * OO:(oc + 1) * OO]).astype(f8)
            for oc in range(OC)]

    # int_sum[n, o]; per core tsq[p, ob, n] with o = ob*128 + p
    powers = np.array(POWERS, dtype=np.float32)
    ts = true_sum.reshape(BATCH, OUT_FEATURES, N_BITS) @ powers  # [B, O]
    tsT = ts.T                                               # [O, B]

    dg = np.eye(128, dtype=np.float32).astype(f8)

    in_maps = []
    for c in range(N_CORES):
        br, oc = c // OC, c % OC
        t = tsT[oc * OO:(oc + 1) * OO, br * NB:(br + 1) * NB]
        tq = np.ascontiguousarray(
            t.reshape(OBLK, 128, NB).transpose(1, 0, 2)).astype(f8)
        in_maps.append({"latq": latqs[br], "wq": nwqs[oc], "tsq": tq,
                        "diag": dg})
    return in_maps


def kernel(latent: np.ndarray, true_sum: np.ndarray,
           weight: np.ndarray) -> np.ndarray:
    from concourse.bass_utils import run_bass_kernel_spmd

    nc = _get_nc()
    in_maps = make_in_maps(latent, true_sum, weight)
    res = run_bass_kernel_spmd(nc, in_maps, list(range(N_CORES)))

    total = 0.0
    for c in range(N_CORES):
        total += float(res.results[c]["partials"].astype(np.float64).sum())
    loss = total / (BATCH * OUT_FEATURES) / (SCALE * SCALE)
    return np.array(loss, dtype=np.float32)
